# revision 1
# baseline (speedup 1.0000x reference)
"""BandSplit kernel for Trainium2 (8 NeuronCores, batch-parallel).

Math (per band i with offset off, width b, K = 2b):
  x[t,k]   : band slice of X, k = re/im-interleaved bins (we reorder k = (c,f))
  z = ((x-mu)*rsqrt(var+eps)*gamma + beta) @ W + bias
    = rsqrt[t] * ( x @ Wg  +  mu[t]*(-colsum)  +  sigma[t]*cvec )
  with Wg = gamma*W (rows), colsum = sum_k Wg[k,:], cvec = beta@W + bias[i],
  sigma = sqrt(var+eps), rsqrt = 1/sigma.

So each output tile [128t, 512d] is ONE accumulation group of f32r matmuls
(lhsT = k-major x rows + a mu row + a sigma row, rhs = augmented W) followed
by a per-partition rsqrt scale fused into the PSUM->SBUF copy.

Per core: batch element b0 = core index. No collectives.
"""
import sys

sys.path.insert(0, "/opt/trn_rl_repo")
import numpy as np

BAND_BINS = [8] * 8 + [16] * 8 + [32] * 8 + [64] * 4 + [128] * 2 + [65]
NB = len(BAND_BINS)  # 31
D = 512
T = 1024
F = sum(BAND_BINS)  # 1025
EPS = 1e-5
NCORES = 8
NJ = T // 128  # 8 t-chunks


def plan():
    """Per-band chunk decomposition. Chunk rows: [x-rows ... , mu, sigma] where
    only the LAST chunk of a band carries the mu/sigma rows (x-rows in a chunk
    always start at tile partition 0). Returns list of bands:
      dict(off, b, chunks=[dict(rows_x, has_ms, wrow0)], ...)
    wrow0 = starting row of this chunk in the W_aug matrix."""
    bands = []
    off = 0
    wrow = 0
    for b in BAND_BINS:
        chunks = []
        if b >= 64:
            # split at the re/im boundary: both chunks compact without any
            # partition-shift DMA (chunk rows start at the c-plane start)
            chunks.append(dict(rows_x=b, has_ms=False, wrow0=wrow, xrow0=0))
            wrow += b
            if b + 2 <= 128:
                chunks.append(dict(rows_x=b, has_ms=True, wrow0=wrow, xrow0=b))
                wrow += b + 2
            else:
                chunks.append(dict(rows_x=b, has_ms=False, wrow0=wrow, xrow0=b))
                wrow += b
                chunks.append(dict(rows_x=0, has_ms=True, wrow0=wrow, xrow0=2 * b))
                wrow += 2
        else:
            chunks.append(dict(rows_x=2 * b, has_ms=True, wrow0=wrow, xrow0=0))
            wrow += 2 * b + 2
        bands.append(dict(off=off, b=b, chunks=chunks))
        off += b
    return bands, wrow


BANDS, W_ROWS = plan()  # W_ROWS == 2112


def build_w_aug(gamma, beta, W, bias):
    """Host-side: build the augmented, per-band-reordered weight matrix.
    k-order inside a band: r = c*b + f  (re plane rows then im plane rows)."""
    w_aug = np.zeros((W_ROWS, D), dtype=np.float32)
    wg = gamma[:, None] * W  # [2F, D]
    for i, bd in enumerate(BANDS):
        off, b = bd["off"], bd["b"]
        s2 = 2 * off
        # band row r -> reference k index
        kidx = np.empty(2 * b, dtype=np.int64)
        kidx[0:b] = s2 + 2 * np.arange(b)      # re rows (c=0)
        kidx[b:2 * b] = s2 + 2 * np.arange(b) + 1  # im rows (c=1)
        xw = wg[kidx]  # [2b, D]
        colsum = xw.sum(axis=0)
        cvec = beta[s2:s2 + 2 * b] @ W[s2:s2 + 2 * b] + bias[i]
        for ch in bd["chunks"]:
            r0, rx = ch["xrow0"], ch["rows_x"]
            w_aug[ch["wrow0"]:ch["wrow0"] + rx] = xw[r0:r0 + rx]
            if ch["has_ms"]:
                w_aug[ch["wrow0"] + rx] = -colsum
                w_aug[ch["wrow0"] + rx + 1] = cvec
    return w_aug


def build_nc():
    import concourse.bacc as bacc
    import concourse.tile as tile
    from concourse import mybir
    from concourse.masks import make_identity

    f32, f32r = mybir.dt.float32, mybir.dt.float32r
    nc = bacc.Bacc(None)
    X = nc.declare_dram_parameter("X", [F, T, 2], f32, isOutput=False)
    WA = nc.declare_dram_parameter("WA", [W_ROWS, D], f32r, isOutput=False)
    OUT = nc.declare_dram_parameter("OUT", [NB, T, D], f32, isOutput=True)

    Xf = X[:].rearrange("f t c -> f (t c)")  # [F, 2048]
    Copy = None  # set below

    with tile.TileContext(nc) as tc:
        with tc.tile_pool(name="consts", bufs=1) as consts, \
             tc.tile_pool(name="nat", bufs=5) as natp, \
             tc.tile_pool(name="kx", bufs=10) as kxp, \
             tc.tile_pool(name="x2", bufs=3) as x2p, \
             tc.tile_pool(name="wp", bufs=16) as wp, \
             tc.tile_pool(name="tmp", bufs=3) as tmpp, \
             tc.tile_pool(name="stat", bufs=8) as statp, \
             tc.tile_pool(name="stage", bufs=6) as stagep, \
             tc.tile_pool(name="pso", bufs=4, space="PSUM") as psop, \
             tc.tile_pool(name="pss", bufs=2, space="PSUM") as pssp, \
             tc.tile_pool(name="psm", bufs=2, space="PSUM") as psmp:

            Copy = mybir.ActivationFunctionType.Copy
            ident = consts.tile([128, 128], f32)
            make_identity(nc, ident)
            ones_f = consts.tile([128, 2], f32)
            nc.vector.memset(ones_f, 1.0)
            ones = consts.tile([128, 2], f32r)
            nc.vector.tensor_scalar_mul(ones, ones_f, 1.0)
            epsc = consts.tile([128, 1], f32)
            nc.vector.memset(epsc, EPS)

            # ---- upfront prefetch of all inputs on the GpSimd (SWDGE) queue;
            # pool bufs throttle how far ahead the loads actually run.
            nats, wtss = [], []
            for bd in BANDS:
                off, b = bd["off"], bd["b"]
                nat = natp.tile([b, 2048], f32, tag="nat")
                nc.gpsimd.dma_start(out=nat, in_=Xf[off:off + b, :])
                nats.append(nat)
                wts = []
                for ch in bd["chunks"]:
                    rows = ch["rows_x"] + (2 if ch["has_ms"] else 0)
                    wt = wp.tile([rows, D], f32r, tag="W")
                    nc.gpsimd.dma_start(
                        out=wt, in_=WA[ch["wrow0"]:ch["wrow0"] + rows, :])
                    wts.append(wt)
                wtss.append(wts)

            eng_flip = [0]

            def flip():
                eng_flip[0] += 1
                return eng_flip[0] % 2 == 0

            def emit_front(i):
                """compaction, stats, musig rows for band i -> state dict"""
                bd = BANDS[i]
                off, b = bd["off"], bd["b"]
                inv_k = 1.0 / (2 * b)
                chunks = bd["chunks"]
                x_chunks = [(ci, ch) for ci, ch in enumerate(chunks)
                            if ch["rows_x"] > 0]
                last_x = len(x_chunks) - 1
                natv = nats[i][:, :].rearrange("f (t c) -> f c t", c=2)

                kxs = []
                for ch in chunks:
                    rows = ch["rows_x"] + (2 if ch["has_ms"] else 0)
                    kx = kxp.tile([rows, T], f32r, tag="kx")
                    kxs.append(kx)

                for ci, ch in enumerate(chunks):
                    r0, rx = ch["xrow0"], ch["rows_x"]
                    if rx == 0:
                        continue
                    r = r0
                    while r < r0 + rx:
                        c, f0 = r // b, r % b
                        f1 = min(b, f0 + (r0 + rx - r))
                        dr = r - r0
                        r += f1 - f0
                        src = natv[f0:f1, c, :]
                        if dr == f0:
                            if flip():
                                nc.vector.tensor_scalar_mul(
                                    kxs[ci][dr:dr + (f1 - f0), :], src, 1.0)
                            else:
                                nc.scalar.activation(
                                    out=kxs[ci][dr:dr + (f1 - f0), :], in_=src,
                                    func=Copy)
                        else:
                            # cross-base shift: widen engine op to partition 0,
                            # then DMA-move the needed rows.
                            srcw = natv[0:f1, c, :]
                            tmp = tmpp.tile([b, T], f32r, tag="imtmp")
                            if flip():
                                nc.vector.tensor_scalar_mul(tmp[0:f1, :], srcw, 1.0)
                            else:
                                nc.scalar.activation(out=tmp[0:f1, :], in_=srcw,
                                                     func=Copy)
                            nc.sync.dma_start(
                                out=kxs[ci][dr:dr + (f1 - f0), :],
                                in_=tmp[f0:f1, :])

                # stats: each accumulation group's matmuls consecutive
                pc = pssp.tile([128, 32], f32, tag="pc")
                x2s = []
                for (ci, ch) in x_chunks:
                    rx = ch["rows_x"]
                    x2 = x2p.tile([rx, T], f32r, tag="x2")
                    if flip():
                        nc.vector.tensor_mul(x2, kxs[ci][0:rx, :], kxs[ci][0:rx, :])
                    else:
                        nc.scalar.activation(out=x2, in_=kxs[ci][0:rx, :],
                                             func=mybir.ActivationFunctionType.Square)
                    x2s.append(x2)
                for j in range(NJ):
                    for xi, (ci, ch) in enumerate(x_chunks):
                        rx = ch["rows_x"]
                        nc.tensor.matmul(pc[:, 2 * j:2 * j + 2],
                                         kxs[ci][0:rx, j * 128:(j + 1) * 128],
                                         ones[0:rx, :],
                                         start=(xi == 0), stop=(xi == last_x))
                for j in range(NJ):
                    for xi, (ci, ch) in enumerate(x_chunks):
                        rx = ch["rows_x"]
                        nc.tensor.matmul(pc[:, 16 + 2 * j:18 + 2 * j],
                                         x2s[xi][:, j * 128:(j + 1) * 128],
                                         ones[0:rx, :],
                                         start=(xi == 0), stop=(xi == last_x))

                # batched stats processing; ms = [mu cols | sigma cols]
                ms = statp.tile([128, 16], f32, tag="ms")
                rs = statp.tile([128, NJ], f32, tag="rs")
                tmpe = statp.tile([128, NJ], f32, tag="tmpe")
                tmpm = statp.tile([128, NJ], f32, tag="tmpm")
                pcx = pc[:, 0:16].rearrange("p (a c) -> p c a", c=2)[:, 0, :]
                pcx2 = pc[:, 16:32].rearrange("p (a c) -> p c a", c=2)[:, 0, :]
                nc.vector.tensor_scalar_mul(ms[:, 0:8], pcx, inv_k)    # mu
                nc.vector.tensor_scalar_mul(tmpe, pcx2, inv_k)         # E[x^2]
                nc.vector.tensor_mul(tmpm, ms[:, 0:8], ms[:, 0:8])     # mu^2
                nc.vector.tensor_sub(tmpe, tmpe, tmpm)                 # var
                nc.scalar.activation(out=ms[:, 8:16], in_=tmpe,
                                     func=mybir.ActivationFunctionType.Sqrt,
                                     bias=epsc, scale=1.0)             # sigma
                nc.vector.reciprocal(out=rs, in_=ms[:, 8:16])          # rsqrt

                # mu/sigma rows via PE transpose + partition-fold DMAs
                mt = psmp.tile([16, 128], f32, tag="mt")
                nc.tensor.transpose(mt, ms, ident)
                mts = statp.tile([16, 128], f32r, tag="mts")
                nc.vector.tensor_scalar_mul(mts, mt, 1.0)
                rem = chunks[-1]["rows_x"]
                kxl = kxs[-1]
                nc.sync.dma_start(
                    out=kxl[rem:rem + 2, :].rearrange("r (j p) -> r j p", j=NJ),
                    in_=mts[:, :])
                return dict(kxs=kxs, rs=rs, chunks=chunks, i=i)

            def emit_back(stt):
                """main matmuls + scale-copy + out DMA for band stt['i']"""
                i, kxs, rs, chunks = stt["i"], stt["kxs"], stt["rs"], stt["chunks"]
                wts = wtss[i]
                for jh in range(2):  # two half-band stages -> 1MB out DMAs
                    stage = stagep.tile([128, NJ // 2, D], f32, tag="stage")
                    for jj in range(NJ // 2):
                        j = jh * (NJ // 2) + jj
                        po = psop.tile([128, D], f32, tag="po")
                        for ci, ch in enumerate(chunks):
                            rows = ch["rows_x"] + (2 if ch["has_ms"] else 0)
                            nc.tensor.matmul(
                                po, kxs[ci][0:rows, j * 128:(j + 1) * 128],
                                wts[ci][0:rows, :],
                                start=(ci == 0), stop=(ci == len(chunks) - 1))
                        if flip():
                            nc.vector.tensor_scalar_mul(stage[:, jj, :], po,
                                                        rs[:, j:j + 1])
                        else:
                            nc.scalar.activation(out=stage[:, jj, :], in_=po,
                                                 func=Copy, scale=rs[:, j:j + 1])
                    nc.sync.dma_start(
                        out=OUT[i, jh * 512:(jh + 1) * 512, :]
                        .rearrange("(j p) d -> p j d", p=128),
                        in_=stage)

            # ---- 2-band software pipeline: front(i) ahead of back(i-2) ----
            from collections import deque
            pend = deque()
            for i in range(NB):
                pend.append(emit_front(i))
                if len(pend) > 4:
                    emit_back(pend.popleft())
            while pend:
                emit_back(pend.popleft())

    nc.finalize()
    return nc


_NC = None


def kernel(X, gamma, beta, W, bias):
    global _NC
    from concourse.bass_utils import run_bass_kernel_spmd

    X = np.asarray(X, dtype=np.float32)
    gamma = np.asarray(gamma, dtype=np.float32)
    beta = np.asarray(beta, dtype=np.float32)
    W = np.asarray(W, dtype=np.float32)
    bias = np.asarray(bias, dtype=np.float32)

    w_aug = build_w_aug(gamma, beta, W, bias)
    if _NC is None:
        _NC = build_nc()
    in_maps = [{"X": X[b], "WA": w_aug} for b in range(NCORES)]
    res = run_bass_kernel_spmd(_NC, in_maps, list(range(NCORES))).results
    return np.stack([res[b]["OUT"] for b in range(NCORES)], axis=0)



# revision 3
# speedup vs baseline: 1.5810x; 1.5810x over previous
"""BandSplit kernel for Trainium2 (8 NeuronCores, batch-parallel), fp16 I/O.

Math (per band i with offset off, width b, K = 2b):
  x[t,k]   : band slice of X, k = re/im-interleaved bins (reordered k = (c,f))
  z = ((x-mu)*rsqrt(var+eps)*gamma + beta) @ W + bias
    = rsqrt[t] * ( x @ Wg  +  mu[t]*(-colsum)  +  sigma[t]*cvec )
  with Wg = gamma*W (rows), colsum = sum_k Wg[k,:], cvec = beta@W + bias[i],
  sigma = sqrt(var+eps), rsqrt = 1/sigma.

All HBM I/O is fp16 (tolerance 2e-2; fp16 keeps rel err ~1e-3):
  X reordered on the host into k-major rows (no on-chip compaction), W
  augmented+reordered on the host, OUT written fp16 and upcast on the host.

Per band, the x rows live in ONE SBUF tile laid out as column blocks of
1024 t-columns; each matmul chunk (K<=128) reads partitions [0:K) of one
block, so a single DMA loads the whole band. mu/sigma rows are folded into
reserved partitions of the tile by a small partition-shift DMA.

Per core: batch element = core index. No collectives.
"""
import sys

sys.path.insert(0, "/opt/trn_rl_repo")
import numpy as np

BAND_BINS = [8] * 8 + [16] * 8 + [32] * 8 + [64] * 4 + [128] * 2 + [65]
NB = len(BAND_BINS)  # 31
D = 512
T = 1024
F = sum(BAND_BINS)  # 1025
EPS = 1e-5
NCORES = 8
NJ = T // 128  # 8 t-chunks


def plan():
    """Per-band layout. Returns list of dicts:
      off, b        : band position
      nxb           : number of 1024-col x blocks in the X tile
      nwb           : number of 512-col blocks in the W tile
      p_x           : partition rows of the X tile
      wrows         : block height of the W tile (equal for all its blocks)
      xr0           : starting row of this band in the reordered X HBM array
      wr0           : starting row of this band in the W HBM array
      xdma_rows     : rows of X HBM loaded (c-major), packed p=xdma_rows//c
      xchunks       : [(blk, k)] x-row chunks for stats (partitions [0:k))
      mains         : [(xblk, wblk, K)] main-matmul chunks
      ms            : (row, colblk) where mu/sigma rows live in the X tile
      sq            : (rows, cols) region to square for stats
    """
    bands = []
    xr = 0
    wr = 0
    for b in BAND_BINS:
        d = dict(b=b, xr0=xr, wr0=wr)
        if b <= 32:
            d.update(nxb=1, nwb=1, p_x=2 * b + 2, wrows=2 * b + 2,
                     xdma_rows=2 * b, xdma_p=2 * b,
                     xchunks=[(0, 2 * b)],
                     mains=[(0, 0, 2 * b + 2)],
                     ms=(2 * b, 0), sq=(2 * b, 1024))
        elif b == 64:
            d.update(nxb=2, nwb=2, p_x=66, wrows=66,
                     xdma_rows=128, xdma_p=64,
                     xchunks=[(0, 64), (1, 64)],
                     mains=[(0, 0, 64), (1, 1, 66)],
                     ms=(64, 1), sq=(64, 2048))
        elif b == 128:
            d.update(nxb=3, nwb=3, p_x=128, wrows=128,
                     xdma_rows=256, xdma_p=128,
                     xchunks=[(0, 128), (1, 128)],
                     mains=[(0, 0, 128), (1, 1, 128), (2, 2, 2)],
                     ms=(0, 2), sq=(128, 2048))
        else:  # b == 65
            d.update(nxb=2, nwb=2, p_x=67, wrows=67,
                     xdma_rows=130, xdma_p=65,
                     xchunks=[(0, 65), (1, 65)],
                     mains=[(0, 0, 65), (1, 1, 67)],
                     ms=(65, 1), sq=(65, 2048))
        d["off"] = sum(BAND_BINS[:len(bands)])
        xr += d["xdma_rows"]
        wr += d["wrows"] * d["nwb"]
        bands.append(d)
    return bands, xr, wr


BANDS, X_ROWS, W_ROWS = plan()  # X_ROWS == 2050


def build_x_perm():
    """Row permutation: X HBM row order is (band; c; f)."""
    perm = np.empty(X_ROWS, dtype=np.int64)
    r = 0
    for bd in BANDS:
        off, b = bd["off"], bd["b"]
        for c in (0, 1):
            perm[r:r + b] = c * F + np.arange(off, off + b)
            r += b
    return perm


X_PERM = build_x_perm()


def build_inputs_host(X, gamma, beta, W, bias):
    """Host-side: reorder X to k-major fp16 rows and build the augmented,
    per-band-blocked fp16 weight matrix."""
    # X: [B, F, T, 2] f32 -> [B, 2*F, T] c-major rows -> per-band order
    Xr = np.moveaxis(X, 3, 1).reshape(X.shape[0], 2 * F, T)
    Xp = np.ascontiguousarray(Xr[:, X_PERM, :]).astype(np.float16)

    w_aug = np.zeros((W_ROWS, D), dtype=np.float32)
    wg = gamma[:, None] * W  # [2F, D]
    for i, bd in enumerate(BANDS):
        off, b = bd["off"], bd["b"]
        s2 = 2 * off
        kidx = np.empty(2 * b, dtype=np.int64)
        kidx[0:b] = s2 + 2 * np.arange(b)          # re rows (c=0)
        kidx[b:2 * b] = s2 + 2 * np.arange(b) + 1  # im rows (c=1)
        xw = wg[kidx]  # [2b, D] in (c, f) order
        colsum = xw.sum(axis=0)
        cvec = beta[s2:s2 + 2 * b] @ W[s2:s2 + 2 * b] + bias[i]
        wr0, h = bd["wr0"], bd["wrows"]
        if bd["nwb"] == 1:
            w_aug[wr0:wr0 + 2 * b] = xw
            w_aug[wr0 + 2 * b] = -colsum
            w_aug[wr0 + 2 * b + 1] = cvec
        elif b in (64, 65):
            w_aug[wr0:wr0 + b] = xw[0:b]                 # blk0: re rows (+pad)
            w_aug[wr0 + h:wr0 + h + b] = xw[b:2 * b]     # blk1: im rows
            w_aug[wr0 + h + b] = -colsum
            w_aug[wr0 + h + b + 1] = cvec
        else:  # b == 128
            w_aug[wr0:wr0 + 128] = xw[0:128]
            w_aug[wr0 + 128:wr0 + 256] = xw[128:256]
            w_aug[wr0 + 256] = -colsum
            w_aug[wr0 + 257] = cvec
    return Xp, w_aug.astype(np.float16)


# per-(band, j) engine for the PSUM->SBUF scaled copy: D=DVE, A=Act
COPY_PATTERN = "DAADAAAA"


def build_nc():
    import concourse.bacc as bacc
    import concourse.tile as tile
    from concourse import mybir
    from concourse.masks import make_identity

    f32, f16 = mybir.dt.float32, mybir.dt.float16
    nc = bacc.Bacc(None)
    XH = nc.declare_dram_parameter("XP", [X_ROWS, T], f16, isOutput=False)
    WH = nc.declare_dram_parameter("WA", [W_ROWS, D], f16, isOutput=False)
    OUT = nc.declare_dram_parameter("OUT", [NB, T, D], f16, isOutput=True)

    with tile.TileContext(nc) as tc:
        with tc.tile_pool(name="consts", bufs=1) as consts, \
             tc.tile_pool(name="xp", bufs=10) as xpool, \
             tc.tile_pool(name="wp", bufs=10) as wpool, \
             tc.tile_pool(name="x2", bufs=4) as x2p, \
             tc.tile_pool(name="stat", bufs=8) as statp, \
             tc.tile_pool(name="stage", bufs=3) as stagep, \
             tc.tile_pool(name="pso", bufs=4, space="PSUM") as psop, \
             tc.tile_pool(name="pss", bufs=2, space="PSUM") as pssp, \
             tc.tile_pool(name="psm", bufs=2, space="PSUM") as psmp:

            Copy = mybir.ActivationFunctionType.Copy
            ident = consts.tile([128, 128], f32)
            make_identity(nc, ident)
            ones = consts.tile([128, 2], f16)
            nc.vector.memset(ones, 1.0)
            epsc = consts.tile([128, 1], f32)
            nc.vector.memset(epsc, EPS)

            # ---- upfront prefetch of all inputs on the GpSimd (SWDGE) queue;
            # pool bufs throttle how far ahead the loads actually run.
            xts, wts = [], []
            for bd in BANDS:
                xt = xpool.tile([bd["p_x"], bd["nxb"] * T], f16, tag="xt")
                xsrc = XH[bd["xr0"]:bd["xr0"] + bd["xdma_rows"], :]
                if bd["nxb"] == 1:
                    nc.gpsimd.dma_start(out=xt[0:bd["xdma_rows"], :], in_=xsrc)
                else:
                    nc.gpsimd.dma_start(
                        out=xt[0:bd["xdma_p"], 0:2 * T].rearrange(
                            "p (c t) -> p c t", c=2),
                        in_=xsrc.rearrange("(c p) t -> p c t", c=2))
                xts.append(xt)
                wt = wpool.tile([bd["wrows"], bd["nwb"] * D], f16, tag="wt")
                rows = bd["wrows"] * bd["nwb"]
                wsrc = WH[bd["wr0"]:bd["wr0"] + rows, :]
                if bd["nwb"] == 1:
                    nc.gpsimd.dma_start(out=wt[:, :], in_=wsrc)
                else:
                    nc.gpsimd.dma_start(
                        out=wt[:, :].rearrange("p (c d) -> p c d", c=bd["nwb"]),
                        in_=wsrc.rearrange("(c p) d -> p c d", c=bd["nwb"]))
                wts.append(wt)

            def emit_front(i):
                """square, stats matmuls, mu/sigma fold for band i"""
                bd = BANDS[i]
                b = bd["b"]
                inv_k = 1.0 / (2 * b)
                xt = xts[i]
                sqr, sqc = bd["sq"]

                x2 = x2p.tile([sqr, sqc], f16, tag="x2")
                nc.vector.tensor_mul(x2, xt[0:sqr, 0:sqc], xt[0:sqr, 0:sqc])

                xchunks = bd["xchunks"]
                last_x = len(xchunks) - 1
                pc = pssp.tile([128, 32], f32, tag="pc")
                for j in range(NJ):
                    for xi, (blk, k) in enumerate(xchunks):
                        c0 = blk * T + j * 128
                        nc.tensor.matmul(pc[:, 2 * j:2 * j + 2],
                                         xt[0:k, c0:c0 + 128],
                                         ones[0:k, :],
                                         start=(xi == 0), stop=(xi == last_x))
                for j in range(NJ):
                    for xi, (blk, k) in enumerate(xchunks):
                        c0 = blk * T + j * 128
                        nc.tensor.matmul(pc[:, 16 + 2 * j:18 + 2 * j],
                                         x2[0:k, c0:c0 + 128],
                                         ones[0:k, :],
                                         start=(xi == 0), stop=(xi == last_x))

                # batched stats processing; ms = [mu cols | sigma cols]
                ms = statp.tile([128, 16], f32, tag="ms")
                rs = statp.tile([128, NJ], f32, tag="rs")
                tmpe = statp.tile([128, NJ], f32, tag="tmpe")
                tmpm = statp.tile([128, NJ], f32, tag="tmpm")
                pcx = pc[:, 0:16].rearrange("p (a c) -> p c a", c=2)[:, 0, :]
                pcx2 = pc[:, 16:32].rearrange("p (a c) -> p c a", c=2)[:, 0, :]
                nc.vector.tensor_scalar_mul(ms[:, 0:8], pcx, inv_k)    # mu
                nc.vector.tensor_scalar_mul(tmpe, pcx2, inv_k)         # E[x^2]
                nc.vector.tensor_mul(tmpm, ms[:, 0:8], ms[:, 0:8])     # mu^2
                nc.vector.tensor_sub(tmpe, tmpe, tmpm)                 # var
                nc.scalar.activation(out=ms[:, 8:16], in_=tmpe,
                                     func=mybir.ActivationFunctionType.Sqrt,
                                     bias=epsc, scale=1.0)             # sigma
                nc.vector.reciprocal(out=rs, in_=ms[:, 8:16])          # rsqrt

                # mu/sigma rows via PE transpose + partition-fold DMA
                mt = psmp.tile([16, 128], f32, tag="mt")
                nc.tensor.transpose(mt, ms, ident)
                mts = statp.tile([16, 128], f16, tag="mts")
                nc.vector.tensor_scalar_mul(mts, mt, 1.0)
                mrow, mblk = bd["ms"]
                nc.sync.dma_start(
                    out=xt[mrow:mrow + 2, mblk * T:(mblk + 1) * T]
                    .rearrange("r (j p) -> r j p", j=NJ),
                    in_=mts[:, :])
                return dict(i=i, rs=rs)

            def emit_back(stt):
                """main matmuls + scale-copy + out DMA for band stt['i']"""
                i, rs = stt["i"], stt["rs"]
                bd = BANDS[i]
                xt, wt = xts[i], wts[i]
                mains = bd["mains"]
                stage = stagep.tile([128, NJ, D], f16, tag="stage")
                for j in range(NJ):
                    po = psop.tile([128, D], f32, tag="po")
                    for ci, (xblk, wblk, K) in enumerate(mains):
                        nc.tensor.matmul(
                            po, xt[0:K, xblk * T + j * 128:xblk * T + (j + 1) * 128],
                            wt[0:K, wblk * D:(wblk + 1) * D],
                            start=(ci == 0), stop=(ci == len(mains) - 1))
                    if COPY_PATTERN[j] == "D":
                        nc.vector.tensor_scalar_mul(stage[:, j, :], po,
                                                    rs[:, j:j + 1])
                    else:
                        nc.scalar.activation(out=stage[:, j, :], in_=po,
                                             func=Copy, scale=rs[:, j:j + 1])
                nc.sync.dma_start(
                    out=OUT[i, :, :].rearrange("(j p) d -> p j d", p=128),
                    in_=stage)

            # ---- software pipeline: front(i) ahead of back(i-4) ----
            from collections import deque
            pend = deque()
            for i in range(NB):
                pend.append(emit_front(i))
                if len(pend) > 4:
                    emit_back(pend.popleft())
            while pend:
                emit_back(pend.popleft())

    nc.finalize()
    return nc


_NC = None


def kernel(X, gamma, beta, W, bias):
    global _NC
    from concourse.bass_utils import run_bass_kernel_spmd

    X = np.asarray(X, dtype=np.float32)
    gamma = np.asarray(gamma, dtype=np.float32)
    beta = np.asarray(beta, dtype=np.float32)
    W = np.asarray(W, dtype=np.float32)
    bias = np.asarray(bias, dtype=np.float32)

    Xp, w_aug = build_inputs_host(X, gamma, beta, W, bias)
    if _NC is None:
        _NC = build_nc()
    in_maps = [{"XP": Xp[b], "WA": w_aug} for b in range(NCORES)]
    res = run_bass_kernel_spmd(_NC, in_maps, list(range(NCORES))).results
    return np.stack([res[b]["OUT"] for b in range(NCORES)], axis=0).astype(
        np.float32)
